# revision 7
# baseline (speedup 1.0000x reference)
"""AttentionBlock (GroupNorm + single-head self-attention + residual) on 8 trn2 cores.

Data-parallel over batch: B=16 -> 2 batch elements per core. Per batch element
(C=512 channels, T=H*W=1024 tokens), everything is kept in channel-major
[C, T] layouts so the whole chain needs zero activation transposes:

  h  = groupnorm(x)                 [C, T]   (bn_stats per channel + block-diag
                                              matmul for cross-partition group agg)
  W  = 64 * wq^T @ wk              [C, C]   (host; x64 so fp8 quant stays in
                                              the normal range, undone in exp scale)
  u  = W^T @ h  (+ 64*gk)           [C, T]
  sT = h^T(j) @ u                   [T, T]   scores transposed: [key j, query i]
  eT = exp(sT * C^-1/2 / 64)        [T, T]   unnormalized softmax numerator
  Z  = ones^T @ eT                  per-query sums, broadcast to 128 partitions
  oT = (v^T @ eT) * (1/Z)           [C, T]   v = h^T @ wv^T
  fT = wo^T' @ oT                   [C, T]
  y  = x + fT + (wo bv + bo)

All matmul operands are fp8 (e4m3), run in DoubleRow perf mode: each PE
instruction contracts 256 rows (two 128-partition k-subtiles addressed via the
pair dim of [128, k, free] tiles) at 0.5 cycles/row. PSUM accumulation is fp32.
Host pre-quantizes the weights to fp8; on-device producers write fp8 directly.
"""

import numpy as np

B, C, HW = 16, 512, 1024
H = W_SP = 32
G = 16  # channels per group (num_groups=32)
NCORES = 8
BL = B // NCORES  # 2 batch elements per core
CT = C // 128  # 4 channel tiles
TT = HW // 128  # 8 token tiles
CH = HW // 512  # 2 free-dim chunks of 512
EPS = 1e-5
SC = float(C) ** -0.5
WS = 64.0  # fp8 pre-scale on W (=wq^T wk) and gk, undone in the exp scale


def build_program(nc, reps=1, fast=True):
    import concourse.bass as bass
    import concourse.tile as tile
    from concourse import mybir

    f32 = mybir.dt.float32
    f8 = mybir.dt.float8e4
    AF = mybir.ActivationFunctionType
    OP = mybir.AluOpType
    DR = mybir.MatmulPerfMode.DoubleRow

    def mm(out, lhsT, rhs, start, stop):
        nc.tensor.matmul(out, lhsT, rhs, start=start, stop=stop, perf_mode=DR)

    x_d = nc.dram_tensor("x", [BL, C, HW], f32, kind="ExternalInput")
    W_d = nc.dram_tensor("Wqk", [C, C], f8, kind="ExternalInput")
    wvT_d = nc.dram_tensor("wvT", [C, C], f8, kind="ExternalInput")
    woT_d = nc.dram_tensor("woT", [C, C], f8, kind="ExternalInput")
    # vecs2[p, k, ci] = kind_k[ci*128+p]; kinds: 0=norm_w 1=norm_b 2=64*gk 3=wob
    vec_d = nc.dram_tensor("vecs2", [128, 4, CT], f32, kind="ExternalInput")
    bd_d = nc.dram_tensor("bd16", [128, 128], f32, kind="ExternalInput")
    y_d = nc.dram_tensor("y", [BL, C, HW], f32, kind="ExternalOutput")

    with tile.TileContext(nc) as tc:
        with (
            tc.tile_pool(name="persist", bufs=1) as persist,
            tc.tile_pool(name="xin", bufs=2) as xin,
            tc.tile_pool(name="big", bufs=2) as big,
            tc.tile_pool(name="yout", bufs=3) as yout,
            tc.tile_pool(name="small", bufs=2) as small,
            tc.tile_pool(name="ps_score", bufs=2, space="PSUM") as ps_score,
            tc.tile_pool(name="ps_acc", bufs=2, space="PSUM") as ps_acc,
        ):
            # ---------------- startup: weights + constants ----------------
            # Batch-0 x first, at the head of the SP queue: the groupnorm
            # chain is the critical path to the first attention matmul.
            x0_t = xin.tile([128, CT, HW], f32, name="x_t")
            for ci in range(CT):
                nc.sync.dma_start(
                    out=x0_t[:, ci, :],
                    in_=x_d[0, ci * 128:(ci + 1) * 128, :],
                )
            # fp8 weights come pre-quantized from the host; gpsimd queue has
            # cheap dma triggers and runs in parallel with the x0 load.
            bd_sb = persist.tile([128, 128], f32)
            nc.gpsimd.dma_start(out=bd_sb, in_=bd_d[:, :])
            vecs = persist.tile([128, 4, CT], f32)
            nc.gpsimd.dma_start(out=vecs, in_=vec_d[:, :, :])
            W_t = persist.tile([128, CT, C], f8)
            wvT_t = persist.tile([128, CT, C], f8)
            woT_t = persist.tile([128, CT, C], f8)
            for ci in range(CT):
                sl = slice(ci * 128, (ci + 1) * 128)
                nc.gpsimd.dma_start(out=wvT_t[:, ci, :], in_=wvT_d[sl, :])
                nc.gpsimd.dma_start(out=W_t[:, ci, :], in_=W_d[sl, :])
                nc.gpsimd.dma_start(out=woT_t[:, ci, :], in_=woT_d[sl, :])
            eps_sb = persist.tile([128, 1], f32)
            nc.vector.memset(eps_sb, EPS)
            ones2 = persist.tile([128, 2, 128], f8)
            nc.vector.memset(ones2, 1.0)

            # ---------------- per batch element ----------------
            for bi, b in enumerate([b for _ in range(reps) for b in range(BL)]):
                if bi == 0:
                    x_t = x0_t
                else:
                    x_t = xin.tile([128, CT, HW], f32, name="x_t")
                    for ci in range(CT):
                        nc.sync.dma_start(
                            out=x_t[:, ci, :],
                            in_=x_d[b, ci * 128:(ci + 1) * 128, :],
                        )

                # --- group norm stats (batched over all 4 channel tiles) ---
                stats = small.tile([128, CT, 2, 6], f32, name="stats")
                for ci in range(CT):
                    for s in range(2):
                        nc.vector.bn_stats(
                            out=stats[:, ci, s, :],
                            in_=x_t[:, ci, s * 512:(s + 1) * 512],
                        )
                mv = small.tile([128, CT, 2], f32, name="mv")
                for ci in range(CT):
                    nc.vector.bn_aggr(out=mv[:, ci, :], in_=stats[:, ci])
                # st2 columns per ci: [mean, E[x^2]]
                st2 = small.tile([128, CT, 2], f32, name="st2")
                nc.vector.tensor_copy(out=st2[:, :, 0:1], in_=mv[:, :, 0:1])
                nc.vector.tensor_mul(out=st2[:, :, 1:2], in0=mv[:, :, 0:1], in1=mv[:, :, 0:1])
                nc.vector.tensor_add(out=st2[:, :, 1:2], in0=st2[:, :, 1:2], in1=mv[:, :, 1:2])
                # block-diag matmul averages the 8 per-partition stat pairs
                # within each 16-channel group (all ci at once)
                ps_st = ps_acc.tile([128, CT, 2], f32, tag="acc", name="ps_st")
                nc.tensor.matmul(ps_st, bd_sb, st2, start=True, stop=True)
                mug = small.tile([128, CT, 1], f32, name="mug")
                nc.vector.tensor_copy(out=mug, in_=ps_st[:, :, 0:1])
                tv = small.tile([128, CT, 1], f32, name="tv")
                nc.vector.tensor_mul(out=tv, in0=mug, in1=mug)
                nc.vector.tensor_sub(out=tv, in0=ps_st[:, :, 1:2], in1=tv)
                nc.scalar.activation(out=tv, in_=tv, func=AF.Sqrt, bias=eps_sb, scale=1.0)
                nc.vector.reciprocal(out=tv, in_=tv)
                sc_c = small.tile([128, CT, 1], f32, name="sc_c")
                nc.vector.tensor_mul(out=sc_c, in0=tv, in1=vecs[:, 0, :].unsqueeze(2))
                bi_c = small.tile([128, CT, 1], f32, name="bi_c")
                nc.vector.tensor_mul(out=bi_c, in0=mug, in1=sc_c)
                nc.vector.tensor_sub(
                    out=bi_c, in0=vecs[:, 1, :].unsqueeze(2), in1=bi_c
                )
                # apply: h = x*sc + bi (fp8 out), then x <- x + wob for the
                # final residual; both on gpsimd (SBUF-only ops)
                h_t = big.tile([128, CT, HW], f8, name="h_t")
                for ci in range(CT):
                    nc.gpsimd.tensor_scalar(
                        out=h_t[:, ci, :], in0=x_t[:, ci, :],
                        scalar1=sc_c[:, ci, :], scalar2=bi_c[:, ci, :],
                        op0=OP.mult, op1=OP.add,
                    )
                    nc.gpsimd.tensor_scalar_add(
                        out=x_t[:, ci, :], in0=x_t[:, ci, :],
                        scalar1=vecs[:, 3, ci:ci + 1],
                    )

                # --- v = h^T @ wv^T  [token, c_out] (fp8, DoubleRow) ---
                v_t = big.tile([128, TT, 512], f8, name="v_t")
                for tp in range(TT // 2):
                    ps_v = ps_acc.tile([128, 2, 512], f32, tag="acc", name="ps_v")
                    for sub in range(2):
                        tt = 2 * tp + sub
                        for cp in range(CT // 2):
                            mm(
                                ps_v[:, sub, :],
                                h_t[:, 2 * cp:2 * cp + 2, tt * 128:(tt + 1) * 128],
                                wvT_t[:, 2 * cp:2 * cp + 2, :],
                                start=(cp == 0), stop=(cp == CT // 2 - 1),
                            )
                    nc.scalar.copy(out=v_t[:, 2 * tp:2 * tp + 2, :], in_=ps_v)

                # --- u = W^T @ h (+64*gk)  [cj, query i] ---
                u_t = big.tile([128, CT, HW], f8, name="u_t")
                for cj in range(CT):
                    ps_u = ps_acc.tile([128, 2, 512], f32, tag="acc", name="ps_u")
                    # ch inner so consecutive matmuls share the stationary
                    # operand (skips the 256-row LDWEIGHTS reload)
                    for cp in range(CT // 2):
                        for ch in range(CH):
                            mm(
                                ps_u[:, ch, :],
                                W_t[:, 2 * cp:2 * cp + 2, cj * 128:(cj + 1) * 128],
                                h_t[:, 2 * cp:2 * cp + 2, ch * 512:(ch + 1) * 512],
                                start=(cp == 0), stop=(cp == CT // 2 - 1),
                            )
                    nc.scalar.activation(
                        out=u_t[:, cj, :], in_=ps_u,
                        func=AF.Identity, bias=vecs[:, 2, cj:cj + 1], scale=1.0,
                    )

                # --- sT = h^T(j) @ u ; eT = exp(sc/WS * sT) ---
                eT_t = big.tile([128, TT, HW], f8, name="eT_t")
                for jt in range(TT):
                    ps_s = ps_score.tile([128, CH, 512], f32, name="ps_s")
                    for cp in range(CT // 2):
                        for ch in range(CH):
                            mm(
                                ps_s[:, ch, :],
                                h_t[:, 2 * cp:2 * cp + 2, jt * 128:(jt + 1) * 128],
                                u_t[:, 2 * cp:2 * cp + 2, ch * 512:(ch + 1) * 512],
                                start=(cp == 0), stop=(cp == CT // 2 - 1),
                            )
                    nc.scalar.activation(
                        out=eT_t[:, jt, :], in_=ps_s, func=AF.Exp, scale=SC / WS,
                    )

                # --- Z = ones^T @ eT (broadcast over partitions), invZ ---
                invZ_t = big.tile([128, HW], f32, name="invZ_t")
                ps_z = ps_acc.tile([128, 2, 512], f32, tag="acc", name="ps_z")
                for ch in range(CH):
                    for tp in range(TT // 2):
                        mm(
                            ps_z[:, ch, :], ones2,
                            eT_t[:, 2 * tp:2 * tp + 2, ch * 512:(ch + 1) * 512],
                            start=(tp == 0), stop=(tp == TT // 2 - 1),
                        )
                nc.vector.reciprocal(out=invZ_t, in_=ps_z)

                # --- oT = (v^T @ eT) * invZ  [c, query i] ---
                oT_t = big.tile([128, CT, HW], f8, name="oT_t")
                for c in range(CT):
                    ps_o = ps_acc.tile([128, 2, 512], f32, tag="acc", name="ps_o")
                    for tp in range(TT // 2):
                        for ch in range(CH):
                            mm(
                                ps_o[:, ch, :],
                                v_t[:, 2 * tp:2 * tp + 2, c * 128:(c + 1) * 128],
                                eT_t[:, 2 * tp:2 * tp + 2, ch * 512:(ch + 1) * 512],
                                start=(tp == 0), stop=(tp == TT // 2 - 1),
                            )
                    nc.vector.tensor_mul(out=oT_t[:, c, :], in0=ps_o, in1=invZ_t)

                # --- fT = woT^T @ oT ; y = (x + wob) + fT ---
                for cp in range(CT):
                    ps_f = ps_acc.tile([128, 2, 512], f32, tag="acc", name="ps_f")
                    for c2 in range(CT // 2):
                        for ch in range(CH):
                            mm(
                                ps_f[:, ch, :],
                                woT_t[:, 2 * c2:2 * c2 + 2, cp * 128:(cp + 1) * 128],
                                oT_t[:, 2 * c2:2 * c2 + 2, ch * 512:(ch + 1) * 512],
                                start=(c2 == 0), stop=(c2 == CT // 2 - 1),
                            )
                    y_t = yout.tile([128, HW], f32, name="y_t")
                    nc.vector.tensor_add(out=y_t, in0=ps_f, in1=x_t[:, cp, :])
                    nc.gpsimd.dma_start(
                        out=y_d[b, cp * 128:(cp + 1) * 128, :], in_=y_t
                    )
    return nc


def _const_inputs():
    bd = np.zeros((128, 128), np.float32)
    for g in range(128 // G):
        bd[g * G:(g + 1) * G, g * G:(g + 1) * G] = 1.0 / G
    return {"bd16": bd}


def prep_inputs(inputs):
    from concourse import mybir

    f8np = np.dtype(mybir.dt.np(mybir.dt.float8e4))
    x = np.ascontiguousarray(np.asarray(inputs["x"], dtype=np.float32)).reshape(B, C, HW)
    wq = np.asarray(inputs["wq"], dtype=np.float32)
    wk = np.asarray(inputs["wk"], dtype=np.float32)
    wv = np.asarray(inputs["wv"], dtype=np.float32)
    wo = np.asarray(inputs["wo"], dtype=np.float32)
    bq = np.asarray(inputs["bq"], dtype=np.float32).reshape(C)
    bv = np.asarray(inputs["bv"], dtype=np.float32).reshape(C)
    bo = np.asarray(inputs["bo"], dtype=np.float32).reshape(C)
    nw = np.asarray(inputs["norm_w"], dtype=np.float32).reshape(C)
    nb = np.asarray(inputs["norm_b"], dtype=np.float32).reshape(C)
    base = dict(_const_inputs())
    base["Wqk"] = np.ascontiguousarray((wq.T @ wk) * WS).astype(f8np)
    base["wvT"] = np.ascontiguousarray(wv.T).astype(f8np)
    base["woT"] = np.ascontiguousarray(wo.T).astype(f8np)
    gk = (wk.T @ bq) * WS
    wob = wo @ bv + bo
    # [C, 4] -> [128, 4 kinds, CT]
    vecs = np.stack([nw, nb, gk, wob], axis=1).reshape(CT, 128, 4).transpose(1, 2, 0)
    base["vecs2"] = np.ascontiguousarray(vecs)
    return base, x


def run_hw(inputs, trace=False):
    from concourse import bacc
    from concourse.bass_utils import run_bass_kernel_spmd

    base, x = prep_inputs(inputs)

    nc = bacc.Bacc("TRN2", target_bir_lowering=False)
    build_program(nc)
    nc.finalize()

    in_maps = [
        {**base, "x": np.ascontiguousarray(x[i * BL:(i + 1) * BL])}
        for i in range(NCORES)
    ]
    try:
        res = run_bass_kernel_spmd(nc, in_maps, list(range(NCORES)), trace=trace)
    except Exception:
        # transient NRT device states (e.g. left over from a prior crashed
        # run) clear on retry
        res = run_bass_kernel_spmd(nc, in_maps, list(range(NCORES)), trace=trace)
    y = np.concatenate([res.results[i]["y"] for i in range(NCORES)], axis=0)
    return y.reshape(B, C, H, W_SP).astype(np.float32), res


def kernel(**inputs):
    y, _ = run_hw(inputs, trace=False)
    return y
